# revision 47
# baseline (speedup 1.0000x reference)
import os
import numpy as np
import ml_dtypes

import concourse.bass as bass
import concourse.tile as tile
from concourse import bacc
from concourse import mybir
from concourse.bass_utils import run_bass_kernel_spmd
from concourse.masks import make_identity

F32 = mybir.dt.float32
F32R = mybir.dt.float32r
BF16 = mybir.dt.bfloat16

D_MODEL, D_STATE, D_CONV = 1024, 16, 4
D_INNER = 2048
B, L = 2, 4096
LN_EPS = 1e-5
N_CORES = 8
TLOC = 1024
KT = D_MODEL // 128
FT = 2 * D_INNER // 128
CT = D_INNER // 128
MT = D_MODEL // 128
NCH = TLOC // 512

_NC_CACHE = None
LAST_RESULT = None


def r32(ap):
    return ap.bitcast(F32R)


def build_graph():
    nc = bacc.Bacc(num_devices=N_CORES)

    xT = nc.declare_dram_parameter("xT", [D_MODEL, TLOC], F32R, isOutput=False)
    wint = nc.declare_dram_parameter("wint", [FT, 128, KT, 128], F32R, isOutput=False)
    wvbt = nc.declare_dram_parameter("wvbt", [128, KT, D_STATE], F32R, isOutput=False)
    cmt = nc.declare_dram_parameter("cmt", [D_STATE, D_INNER], F32R, isOutput=False)
    wot = nc.declare_dram_parameter("wot", [MT, 128, CT, 128], BF16, isOutput=False)
    convw = nc.declare_dram_parameter("convw", [128, CT, D_CONV], F32, isOutput=False)
    convb = nc.declare_dram_parameter("convb", [128, CT], F32, isOutput=False)
    biasz = nc.declare_dram_parameter("biasz", [128, CT], F32, isOutput=False)
    uhalo = nc.declare_dram_parameter("uhalo", [128, CT, 3], BF16, isOutput=False)
    vadd = nc.declare_dram_parameter("vadd", [D_STATE, TLOC], F32, isOutput=False)
    decay = nc.declare_dram_parameter("decay", [D_STATE, 1], F32, isOutput=False)
    gct = nc.declare_dram_parameter("gct", [D_STATE, N_CORES], F32, isOutput=False)
    res = nc.declare_dram_parameter("res", [D_MODEL, TLOC], F32, isOutput=True)

    with tile.TileContext(nc) as tc:
        with (
            tc.tile_pool(name="sb", bufs=1) as sb,
            tc.tile_pool(name="sb2", bufs=2) as sb2,
            tc.tile_pool(name="ps", bufs=5, space="PSUM") as ps,
            tc.tile_pool(name="pss", bufs=3, space="PSUM") as pss,
            tc.tile_pool(name="dr", bufs=1, space="DRAM") as dr,
        ):
            with nc.allow_low_precision(reason="f32r matmul pipeline"):
                _emit(nc, tc, sb, sb2, ps, pss, dr, locals())
    nc.compile()
    return nc


def _emit(nc, tc, sb, sb2, ps, pss, dr, t):
    xT, wint, wvbt, cmt, wot = t["xT"], t["wint"], t["wvbt"], t["cmt"], t["wot"]
    convw, convb, biasz = t["convw"], t["convb"], t["biasz"]
    uhalo, vadd, decay, gct, res = t["uhalo"], t["vadd"], t["decay"], t["gct"], t["res"]

    x_sb = sb.tile([128, KT, TLOC], F32R)
    xTr = xT.rearrange("(ko ki) t -> ki ko t", ki=128)
    for n in range(NCH):
        for ko in range(KT):
            nc.sync.dma_start(out=x_sb[:, ko, n * 512 : (n + 1) * 512],
                              in_=xTr[:, ko, n * 512 : (n + 1) * 512])

    ones_k0 = sb.tile([128, 1], F32)
    nc.vector.memset(ones_k0, 1.0 / D_MODEL)
    ones_k = sb.tile([128, 1], F32R)
    nc.vector.tensor_copy(ones_k, ones_k0)
    ones_b = sb.tile([1, 128], F32)
    nc.vector.memset(ones_b, 1.0)
    eps_t = sb.tile([1, 1], F32)
    nc.vector.memset(eps_t, LN_EPS)
    ident = sb.tile([128, 128], BF16)
    make_identity(nc, ident)
    convw_sb = sb.tile([128, CT, D_CONV], F32)
    nc.sync.dma_start(out=convw_sb, in_=t["convw"][:])
    convb_sb = sb.tile([128, CT], F32)
    nc.sync.dma_start(out=convb_sb, in_=convb[:])
    biasz_sb = sb.tile([128, CT], F32)
    nc.sync.dma_start(out=biasz_sb, in_=biasz[:])
    rows = sb.tile([1, 3 * TLOC], F32)
    states = sb.tile([D_STATE, 3 * TLOC], F32)
    s_sb = sb.tile([D_STATE, TLOC], F32R)
    vadd_sb = states[:, 2 * TLOC : 3 * TLOC]
    nc.sync.dma_start(out=vadd_sb, in_=vadd[:])
    decay_c = sb.tile([D_STATE, 1], F32)
    nc.sync.dma_start(out=decay_c, in_=decay[:])
    gct_sb = sb.tile([D_STATE, N_CORES], F32)
    nc.sync.dma_start(out=gct_sb, in_=gct[:])

    mu_row = rows[:, 0:TLOC]
    var_row = rows[:, TLOC : 2 * TLOC]
    mu2_row = rows[:, 2 * TLOC : 3 * TLOC]
    r_row = mu2_row
    mur_row = var_row
    rb_sb = sb.tile([128, TLOC], F32)
    murb_sb = sb.tile([128, TLOC], F32)

    sq_half = []
    for n in range(NCH):
        cs = slice(n * 512, (n + 1) * 512)
        mu_ps = pss.tile([1, 512], F32, tag="sm", name=f"mu_ps{n}")
        sq_ps = pss.tile([1, 512], F32, tag="sm", name=f"sq_ps{n}")
        for ko in range(KT):
            sq_scr = sb2.tile([128, 512], F32R, name="sq_scr", bufs=1)
            nc.scalar.square(sq_scr, x_sb[:, ko, cs])
            nc.tensor.matmul(mu_ps, ones_k, x_sb[:, ko, cs],
                             start=(ko == 0), stop=(ko == KT - 1))
            nc.tensor.matmul(sq_ps, ones_k, sq_scr,
                             start=(ko == 0), stop=(ko == KT - 1))
        nc.vector.tensor_copy(mu_row[:, cs], mu_ps)
        nc.vector.tensor_copy(var_row[:, cs], sq_ps)
        nc.vector.tensor_mul(mu2_row[:, cs], mu_row[:, cs], mu_row[:, cs])
        nc.vector.tensor_sub(var_row[:, cs], var_row[:, cs], mu2_row[:, cs])
        nc.scalar.activation(var_row[:, cs], var_row[:, cs],
                             mybir.ActivationFunctionType.Sqrt,
                             bias=eps_t, scale=1.0)
        nc.vector.reciprocal_approx_fast(r_row[:, cs], var_row[:, cs])
        nc.vector.tensor_mul(mur_row[:, cs], mu_row[:, cs], r_row[:, cs])

        for srow, dst in ((r_row, rb_sb), (mur_row, murb_sb)):
            b_ps = pss.tile([128, 512], F32, tag="sm", name="b_ps")
            nc.tensor.matmul(b_ps, ones_b, srow[:, cs], start=True, stop=True)
            nc.vector.tensor_copy(dst[:, cs], b_ps)

        for ko in range(KT):
            nc.vector.tensor_mul(x_sb[:, ko, cs], x_sb[:, ko, cs], rb_sb[:, cs])
            nc.vector.tensor_sub(x_sb[:, ko, cs], x_sb[:, ko, cs], murb_sb[:, cs])

    wvb_sb = sb.tile([128, KT, D_STATE], F32R)
    nc.sync.dma_start(out=wvb_sb, in_=wvbt[:])
    v_sb = states[:, 0:TLOC]
    for n in range(NCH):
        cs = slice(n * 512, (n + 1) * 512)
        v_ps = pss.tile([D_STATE, 512], F32, tag="sm", name="v_ps")
        for ko in range(KT):
            nc.tensor.matmul(v_ps, wvb_sb[:, ko, :], x_sb[:, ko, cs],
                             start=(ko == 0), stop=(ko == KT - 1))
        nc.vector.tensor_add(v_sb[:, cs], v_ps, vadd_sb[:, cs])

    decay_t = states[:, TLOC : 2 * TLOC]
    nc.vector.memset(decay_t, 1.0)
    nc.vector.tensor_scalar_mul(decay_t, in0=decay_t, scalar1=decay_c)

    l_sb = vadd_sb
    nc.vector.tensor_tensor_scan(l_sb, decay_t, v_sb, 0.0,
                                 mybir.AluOpType.mult, mybir.AluOpType.add)

    cc_in = dr.tile([D_STATE, 1], F32)
    cc_out = dr.tile([D_STATE * N_CORES, 1], F32, addr_space="Shared")
    nc.sync.dma_start(out=cc_in[:], in_=l_sb[:, TLOC - 1 : TLOC])
    nc.gpsimd.collective_compute(
        "AllGather", mybir.AluOpType.bypass,
        replica_groups=[list(range(N_CORES))],
        ins=[cc_in[:]], outs=[cc_out[:]],
    )
    lam_all = sb.tile([D_STATE, N_CORES], F32)
    nc.sync.dma_start(out=lam_all,
                      in_=cc_out.rearrange("(j d) one -> d (j one)", d=D_STATE))
    sig_scr = sb.tile([D_STATE, N_CORES], F32)
    sigma = sb.tile([D_STATE, 1], F32)
    nc.vector.scalar_tensor_tensor(
        out=sig_scr, in0=lam_all, scalar=1.0, in1=gct_sb,
        op0=mybir.AluOpType.mult, op1=mybir.AluOpType.mult, accum_out=sigma)
    nc.vector.tensor_tensor_scan(s_sb, decay_t, v_sb, sigma,
                                 mybir.AluOpType.mult, mybir.AluOpType.add)

    u_sb = sb.tile([128, CT, TLOC + 3], BF16)
    nc.sync.dma_start(out=u_sb[:, :, 0:3], in_=uhalo[:])
    sigz_sb = sb.tile([128, CT, TLOC], BF16)
    for f in range(FT):
        wt = sb2.tile([128, KT, 128], F32R, name="wt", bufs=3)
        nc.sync.dma_start(out=wt, in_=wint[f])
        for n in range(NCH):
            cs = slice(n * 512, (n + 1) * 512)
            p_t = ps.tile([128, 512], F32, tag="mm", name=f"ip{f}_{n}")
            for ko in range(KT):
                nc.tensor.matmul(p_t, wt[:, ko, :], x_sb[:, ko, cs],
                                 start=(ko == 0), stop=(ko == KT - 1))
            if f < CT:
                nc.scalar.copy(
                    out=u_sb[:, f, 3 + n * 512 : 3 + (n + 1) * 512],
                    in_=p_t)
            else:
                c = f - CT
                nc.scalar.activation(
                    out=sigz_sb[:, c, n * 512 : (n + 1) * 512],
                    in_=p_t, func=mybir.ActivationFunctionType.Sigmoid,
                    bias=biasz_sb[:, c : c + 1], scale=1.0)

    cmt_sb = sb.tile([D_STATE, D_INNER], F32R)
    nc.sync.dma_start(out=cmt_sb, in_=cmt[:])
    y_sb = sb.tile([128, CT, TLOC], BF16)
    for c in range(CT):
        acc = sb2.tile([128, TLOC], F32, name="cacc", bufs=2)
        nc.vector.tensor_scalar_mul(
            out=acc, in0=u_sb[:, c, 0:TLOC], scalar1=convw_sb[:, c, 0:1])
        for tap in range(1, D_CONV):
            nc.vector.scalar_tensor_tensor(
                out=acc, in0=u_sb[:, c, tap : tap + TLOC],
                scalar=convw_sb[:, c, tap : tap + 1], in1=acc,
                op0=mybir.AluOpType.mult, op1=mybir.AluOpType.add)
        nc.scalar.activation(
            out=u_sb[:, c, 3 : 3 + TLOC], in_=acc,
            func=mybir.ActivationFunctionType.Silu,
            bias=convb_sb[:, c : c + 1], scale=1.0)
        for n in range(NCH):
            cs = slice(n * 512, (n + 1) * 512)
            sc_ps = ps.tile([128, 512], F32, tag="mm", name=f"sc{c}_{n}")
            nc.tensor.matmul(sc_ps, cmt_sb[:, c * 128 : (c + 1) * 128],
                             s_sb[:, cs], start=True, stop=False)
            nc.tensor.matmul(sc_ps, ident,
                             u_sb[:, c, 3 + n * 512 : 3 + (n + 1) * 512],
                             start=False, stop=True)
            nc.vector.tensor_mul(y_sb[:, c, cs], sc_ps, sigz_sb[:, c, cs])

    for m in range(MT):
        wo = sb2.tile([128, CT, 128], BF16, name="wo", bufs=2)
        nc.sync.dma_start(out=wo, in_=wot[m])
        for n in range(NCH):
            cs = slice(n * 512, (n + 1) * 512)
            o_ps = ps.tile([128, 512], F32, tag="mm", name=f"op{m}_{n}")
            for c in range(CT):
                nc.tensor.matmul(o_ps, wo[:, c, :], y_sb[:, c, cs],
                                 start=(c == 0), stop=(c == CT - 1))
            r_sb = sb2.tile([128, 512], F32, name="r_sb", bufs=1)
            nc.scalar.copy(r_sb, o_ps)
            nc.sync.dma_start(out=res[m * 128 : (m + 1) * 128, cs], in_=r_sb)



def _standardize(x):
    mu = x.mean(-1, keepdims=True)
    var = ((x - mu) ** 2).mean(-1, keepdims=True)
    return ((x - mu) / np.sqrt(var + LN_EPS)).astype(np.float32)


def host_prepare(inputs):
    x = np.ascontiguousarray(np.asarray(inputs["x"], np.float32))
    g = np.asarray(inputs["ln_gamma"], np.float32)
    beta = np.asarray(inputs["ln_beta"], np.float32)
    W_in = np.asarray(inputs["W_in"], np.float32)
    conv_w = np.asarray(inputs["conv_w"], np.float32)[:, 0, :]
    conv_b = np.asarray(inputs["conv_b"], np.float32)
    W_out = np.asarray(inputs["W_out"], np.float32)
    A = np.asarray(inputs["A"], np.float32)
    Bm = np.asarray(inputs["Bm"], np.float32)
    Cm = np.asarray(inputs["Cm"], np.float32)

    Wg = W_in * g[None, :]
    b_in = W_in @ beta
    bias_u = b_in[:D_INNER]
    bias_z = b_in[D_INNER:]
    W1g = Wg[:D_INNER]

    Wvb0 = (Bm @ W_in[:D_INNER]) * g[None, :]
    bias_v0 = Bm @ W_in[:D_INNER] @ beta

    fallback = False
    lamc, V = np.linalg.eig(A.astype(np.float64))
    if np.abs(lamc.imag).max() > 1e-9 or np.linalg.cond(V) > 1e3:
        fallback = True
    if fallback:
        lam = np.zeros(D_STATE, np.float32)
        Wvb = np.zeros_like(Wvb0)
        Cmt = Cm.astype(np.float32)
        xn = _standardize(x.reshape(-1, D_MODEL)).reshape(x.shape) * g + beta
        v = xn.astype(np.float32) @ (Bm @ W_in[:D_INNER]).T
        sT = np.zeros((B, L, D_STATE), np.float32)
        for b_ in range(B):
            cur = np.zeros(D_STATE, np.float64)
            Ad = A.astype(np.float64)
            for tt in range(L):
                cur = Ad @ cur + v[b_, tt]
                sT[b_, tt] = cur
        sT = np.nan_to_num(sT, posinf=3e38, neginf=-3e38)
    else:
        lam = lamc.real
        Vr = V.real
        Vi = np.linalg.inv(Vr)
        Wvb = (Vi @ Wvb0).astype(np.float32)
        bias_vt = (Vi @ bias_v0).astype(np.float32)
        Cmt = (Vr.T @ Cm).astype(np.float32)

    wint = np.ascontiguousarray(
        Wg.reshape(FT, 128, KT, 128).transpose(0, 3, 2, 1))
    wvbt = np.ascontiguousarray(
        Wvb.reshape(D_STATE, KT, 128).transpose(2, 1, 0)) if not fallback \
        else np.zeros((128, KT, D_STATE), np.float32)
    wot = np.ascontiguousarray(
        W_out.reshape(MT, 128, CT, 128).transpose(0, 3, 2, 1)
    ).astype(ml_dtypes.bfloat16)
    convw_p = np.ascontiguousarray(conv_w.reshape(CT, 128, D_CONV).transpose(1, 0, 2))
    convb_f = conv_b + bias_u * conv_w.sum(axis=1)
    convb_p = np.ascontiguousarray(convb_f.reshape(CT, 128).T)
    biasz_p = np.ascontiguousarray(bias_z.reshape(CT, 128).T)
    decay_p = lam.astype(np.float32).reshape(D_STATE, 1)

    in_maps = []
    for c in range(N_CORES):
        b_, k = c // 4, c % 4
        xs = x[b_, k * TLOC : (k + 1) * TLOC]
        xTc = np.ascontiguousarray(xs.T)

        if k == 0:
            uh = np.zeros((D_INNER, 3), np.float32)
        else:
            xh = x[b_, k * TLOC - 3 : k * TLOC]
            uh = (_standardize(xh) @ W1g.T).T
        uh_p = np.ascontiguousarray(
            uh.reshape(CT, 128, 3).transpose(1, 0, 2)).astype(ml_dtypes.bfloat16)

        if fallback:
            va = np.ascontiguousarray(sT[b_, k * TLOC : (k + 1) * TLOC].T)
            G = np.zeros((N_CORES, D_STATE), np.float32)
        else:
            va = np.broadcast_to(bias_vt[:, None], (D_STATE, TLOC)).copy()
            G = np.zeros((N_CORES, D_STATE), np.float32)
            for j in range(N_CORES):
                bj, kj = j // 4, j % 4
                if bj == b_ and kj < k:
                    G[j] = lam ** (TLOC * (k - kj))
        in_maps.append(dict(
            xT=xTc, wint=wint, wvbt=wvbt, cmt=Cmt.astype(np.float32),
            wot=wot, convw=convw_p, convb=convb_p,
            biasz=biasz_p, uhalo=uh_p, vadd=va.astype(np.float32),
            decay=decay_p, gct=np.ascontiguousarray(G.T),
        ))
    return in_maps, x


def get_nc():
    global _NC_CACHE
    if _NC_CACHE is None:
        _NC_CACHE = build_graph()
    return _NC_CACHE


def kernel(**inputs):
    global LAST_RESULT
    nc = get_nc()
    in_maps, x = host_prepare(inputs)
    trace = bool(os.environ.get("BASS_TRACE"))
    r = run_bass_kernel_spmd(nc, in_maps, core_ids=list(range(N_CORES)),
                             trace=trace)
    LAST_RESULT = r
    out = np.empty((B, L, D_MODEL), np.float32)
    for c in range(N_CORES):
        b_, k = c // 4, c % 4
        resT = r.results[c]["res"]
        out[b_, k * TLOC : (k + 1) * TLOC] = (
            x[b_, k * TLOC : (k + 1) * TLOC] + resT.T)
    return out


# revision 60
# speedup vs baseline: 1.0095x; 1.0095x over previous
import os
import numpy as np
import ml_dtypes

import concourse.bass as bass
import concourse.tile as tile
from concourse import bacc
from concourse import mybir
from concourse.bass_utils import run_bass_kernel_spmd
from concourse.masks import make_identity

F32 = mybir.dt.float32
F32R = mybir.dt.float32r
BF16 = mybir.dt.bfloat16

D_MODEL, D_STATE, D_CONV = 1024, 16, 4
D_INNER = 2048
B, L = 2, 4096
LN_EPS = 1e-5
N_CORES = 8
TLOC = 1024
KT = D_MODEL // 128
FT = 2 * D_INNER // 128
CT = D_INNER // 128
MT = D_MODEL // 128
NCH = TLOC // 512

_NC_CACHE = None
LAST_RESULT = None


def r32(ap):
    return ap.bitcast(F32R)


def build_graph():
    nc = bacc.Bacc(num_devices=N_CORES)

    xT = nc.declare_dram_parameter("xT", [D_MODEL, TLOC], F32R, isOutput=False)
    wint = nc.declare_dram_parameter("wint", [FT, 128, KT, 128], F32R, isOutput=False)
    wvbt = nc.declare_dram_parameter("wvbt", [128, KT, D_STATE], F32R, isOutput=False)
    cmt = nc.declare_dram_parameter("cmt", [D_STATE, D_INNER], F32R, isOutput=False)
    wot = nc.declare_dram_parameter("wot", [MT, 128, CT, 128], BF16, isOutput=False)
    convw = nc.declare_dram_parameter("convw", [128, CT, D_CONV], F32, isOutput=False)
    convb = nc.declare_dram_parameter("convb", [128, CT], F32, isOutput=False)
    biasz = nc.declare_dram_parameter("biasz", [128, CT], F32, isOutput=False)
    uhalo = nc.declare_dram_parameter("uhalo", [128, CT, 3], BF16, isOutput=False)
    vadd = nc.declare_dram_parameter("vadd", [D_STATE, TLOC], F32, isOutput=False)
    decay = nc.declare_dram_parameter("decay", [D_STATE, 1], F32, isOutput=False)
    gct = nc.declare_dram_parameter("gct", [D_STATE, N_CORES], F32, isOutput=False)
    res = nc.declare_dram_parameter("res", [D_MODEL, TLOC], F32, isOutput=True)

    with tile.TileContext(nc) as tc:
        with (
            tc.tile_pool(name="sb", bufs=1) as sb,
            tc.tile_pool(name="sb2", bufs=2) as sb2,
            tc.tile_pool(name="ps", bufs=6, space="PSUM") as ps,
            tc.tile_pool(name="pss", bufs=2, space="PSUM") as pss,
            tc.tile_pool(name="dr", bufs=1, space="DRAM") as dr,
        ):
            with nc.allow_low_precision(reason="f32r matmul pipeline"):
                _emit(nc, tc, sb, sb2, ps, pss, dr, locals())
    nc.compile()
    return nc


def _emit(nc, tc, sb, sb2, ps, pss, dr, t):
    xT, wint, wvbt, cmt, wot = t["xT"], t["wint"], t["wvbt"], t["cmt"], t["wot"]
    convw, convb, biasz = t["convw"], t["convb"], t["biasz"]
    uhalo, vadd, decay, gct, res = t["uhalo"], t["vadd"], t["decay"], t["gct"], t["res"]

    x_sb = sb.tile([128, KT, TLOC], F32R)
    xTr = xT.rearrange("(ko ki) t -> ki ko t", ki=128)
    for n in range(NCH):
        for ko in range(KT):
            nc.sync.dma_start(out=x_sb[:, ko, n * 512 : (n + 1) * 512],
                              in_=xTr[:, ko, n * 512 : (n + 1) * 512])

    ones_k0 = sb.tile([128, 1], F32)
    nc.vector.memset(ones_k0, 1.0 / D_MODEL)
    ones_k = sb.tile([128, 1], F32R)
    nc.vector.tensor_copy(ones_k, ones_k0)
    ones_b = sb.tile([1, 128], F32)
    nc.vector.memset(ones_b, 1.0)
    eps_t = sb.tile([1, 1], F32)
    nc.vector.memset(eps_t, LN_EPS)
    ident = sb.tile([128, 128], BF16)
    make_identity(nc, ident)
    convw_sb = sb.tile([128, CT, D_CONV], F32)
    nc.sync.dma_start(out=convw_sb, in_=t["convw"][:])
    convb_sb = sb.tile([128, CT], F32)
    nc.sync.dma_start(out=convb_sb, in_=convb[:])
    biasz_sb = sb.tile([128, CT], F32)
    nc.sync.dma_start(out=biasz_sb, in_=biasz[:])
    rows = sb.tile([1, 2 * TLOC + 512], F32)
    states = sb.tile([D_STATE, 2 * TLOC], F32)
    s_sb = sb.tile([D_STATE, TLOC], F32R)
    vadd_sb = states[:, TLOC : 2 * TLOC]
    nc.sync.dma_start(out=vadd_sb, in_=vadd[:])
    decay_c = sb.tile([D_STATE, 1], F32)
    nc.sync.dma_start(out=decay_c, in_=decay[:])
    gct_sb = sb.tile([D_STATE, N_CORES], F32)
    nc.sync.dma_start(out=gct_sb, in_=gct[:])

    mu_row = rows[:, 0:TLOC]
    var_row = rows[:, TLOC : 2 * TLOC]
    mu2_row = rows[:, 2 * TLOC : 2 * TLOC + 512]
    r_row = mu2_row
    mur_row = var_row
    rb_sb = sb.tile([128, TLOC], F32)
    murb_sb = sb.tile([128, TLOC], F32)

    sq_half = []
    for n in range(NCH):
        cs = slice(n * 512, (n + 1) * 512)
        mu_ps = pss.tile([1, 512], F32, tag="sm", name=f"mu_ps{n}")
        sq_ps = pss.tile([1, 512], F32, tag="sm", name=f"sq_ps{n}")
        for ko in range(KT):
            sq_scr = sb2.tile([128, 512], F32R, name="sq_scr", bufs=2)
            nc.scalar.square(sq_scr, x_sb[:, ko, cs])
            nc.tensor.matmul(mu_ps, ones_k, x_sb[:, ko, cs],
                             start=(ko == 0), stop=(ko == KT - 1))
            nc.tensor.matmul(sq_ps, ones_k, sq_scr,
                             start=(ko == 0), stop=(ko == KT - 1))
        nc.vector.tensor_copy(mu_row[:, cs], mu_ps)
        nc.vector.tensor_copy(var_row[:, cs], sq_ps)
        nc.vector.tensor_mul(mu2_row, mu_row[:, cs], mu_row[:, cs])
        nc.vector.tensor_sub(var_row[:, cs], var_row[:, cs], mu2_row)
        nc.scalar.activation(var_row[:, cs], var_row[:, cs],
                             mybir.ActivationFunctionType.Sqrt,
                             bias=eps_t, scale=1.0)
        nc.vector.reciprocal_approx_fast(mu2_row, var_row[:, cs])
        r_half = mu2_row
        nc.vector.tensor_mul(mur_row[:, cs], mu_row[:, cs], r_half)

        for srow, dst in ((r_half, rb_sb), (mur_row[:, cs], murb_sb)):
            b_ps = pss.tile([128, 512], F32, tag="sm", name="b_ps")
            nc.tensor.matmul(b_ps, ones_b, srow, start=True, stop=True)
            nc.vector.tensor_copy(dst[:, cs], b_ps)

        for ko in range(KT):
            nc.vector.tensor_mul(x_sb[:, ko, cs], x_sb[:, ko, cs], rb_sb[:, cs])
            nc.vector.tensor_sub(x_sb[:, ko, cs], x_sb[:, ko, cs], murb_sb[:, cs])

    wvb_sb = sb.tile([128, KT, D_STATE], F32R)
    nc.sync.dma_start(out=wvb_sb, in_=wvbt[:])
    v_sb = states[:, 0:TLOC]
    for n in range(NCH):
        cs = slice(n * 512, (n + 1) * 512)
        v_ps = pss.tile([D_STATE, 512], F32, tag="sm", name="v_ps")
        for ko in range(KT):
            nc.tensor.matmul(v_ps, wvb_sb[:, ko, :], x_sb[:, ko, cs],
                             start=(ko == 0), stop=(ko == KT - 1))
        nc.vector.tensor_add(v_sb[:, cs], v_ps, vadd_sb[:, cs])

    decay_t = decay_c.broadcast_to([D_STATE, TLOC])

    l_sb = vadd_sb
    nc.vector.tensor_tensor_scan(l_sb, decay_t, v_sb, 0.0,
                                 mybir.AluOpType.mult, mybir.AluOpType.add)

    cc_in = dr.tile([D_STATE, 1], F32)
    cc_out = dr.tile([D_STATE * N_CORES, 1], F32, addr_space="Shared")
    nc.sync.dma_start(out=cc_in[:], in_=l_sb[:, TLOC - 1 : TLOC])
    nc.gpsimd.collective_compute(
        "AllGather", mybir.AluOpType.bypass,
        replica_groups=[list(range(N_CORES))],
        ins=[cc_in[:]], outs=[cc_out[:]],
    )
    lam_all = sb.tile([D_STATE, N_CORES], F32)
    nc.sync.dma_start(out=lam_all,
                      in_=cc_out.rearrange("(j d) one -> d (j one)", d=D_STATE))
    sig_scr = sb.tile([D_STATE, N_CORES], F32)
    sigma = sb.tile([D_STATE, 1], F32)
    nc.vector.scalar_tensor_tensor(
        out=sig_scr, in0=lam_all, scalar=1.0, in1=gct_sb,
        op0=mybir.AluOpType.mult, op1=mybir.AluOpType.mult, accum_out=sigma)
    nc.vector.tensor_tensor_scan(s_sb, decay_t, v_sb, sigma,
                                 mybir.AluOpType.mult, mybir.AluOpType.add)

    u_sb = sb.tile([128, CT, TLOC + 3], BF16)
    nc.sync.dma_start(out=u_sb[:, :, 0:3], in_=uhalo[:])
    sigz_sb = sb.tile([128, CT, TLOC], BF16)
    for f in range(FT):
        wt = sb2.tile([128, KT, 128], F32R, name="wt", bufs=4)
        nc.sync.dma_start(out=wt, in_=wint[f])
        for n in range(NCH):
            cs = slice(n * 512, (n + 1) * 512)
            p_t = ps.tile([128, 512], F32, tag="mm", name=f"ip{f}_{n}")
            for ko in range(KT):
                nc.tensor.matmul(p_t, wt[:, ko, :], x_sb[:, ko, cs],
                                 start=(ko == 0), stop=(ko == KT - 1))
            if f < CT:
                nc.scalar.copy(
                    out=u_sb[:, f, 3 + n * 512 : 3 + (n + 1) * 512],
                    in_=p_t)
            else:
                c = f - CT
                nc.scalar.activation(
                    out=sigz_sb[:, c, n * 512 : (n + 1) * 512],
                    in_=p_t, func=mybir.ActivationFunctionType.Sigmoid,
                    bias=biasz_sb[:, c : c + 1], scale=1.0)

    cmt_sb = sb.tile([D_STATE, D_INNER], F32R)
    nc.sync.dma_start(out=cmt_sb, in_=cmt[:])
    y_sb = sb.tile([128, CT, TLOC], BF16)
    for c in range(CT):
        acc = sb2.tile([128, TLOC], F32, name="cacc", bufs=2)
        nc.vector.tensor_scalar_mul(
            out=acc, in0=u_sb[:, c, 0:TLOC], scalar1=convw_sb[:, c, 0:1])
        for tap in range(1, D_CONV):
            nc.vector.scalar_tensor_tensor(
                out=acc, in0=u_sb[:, c, tap : tap + TLOC],
                scalar=convw_sb[:, c, tap : tap + 1], in1=acc,
                op0=mybir.AluOpType.mult, op1=mybir.AluOpType.add)
        nc.scalar.activation(
            out=u_sb[:, c, 3 : 3 + TLOC], in_=acc,
            func=mybir.ActivationFunctionType.Silu,
            bias=convb_sb[:, c : c + 1], scale=1.0)
        for n in range(NCH):
            cs = slice(n * 512, (n + 1) * 512)
            sc_ps = ps.tile([128, 512], F32, tag="mm", name=f"sc{c}_{n}")
            nc.tensor.matmul(sc_ps, cmt_sb[:, c * 128 : (c + 1) * 128],
                             s_sb[:, cs], start=True, stop=False)
            nc.tensor.matmul(sc_ps, ident,
                             u_sb[:, c, 3 + n * 512 : 3 + (n + 1) * 512],
                             start=False, stop=True)
            nc.vector.tensor_mul(y_sb[:, c, cs], sc_ps, sigz_sb[:, c, cs])

    for m in range(MT):
        wo = sb2.tile([128, CT, 128], BF16, name="wo", bufs=2)
        nc.sync.dma_start(out=wo, in_=wot[m])
        for n in range(NCH):
            cs = slice(n * 512, (n + 1) * 512)
            o_ps = ps.tile([128, 512], F32, tag="mm", name=f"op{m}_{n}")
            for c in range(CT):
                nc.tensor.matmul(o_ps, wo[:, c, :], y_sb[:, c, cs],
                                 start=(c == 0), stop=(c == CT - 1))
            r_sb = sb2.tile([128, 512], F32, name="r_sb", bufs=1)
            nc.scalar.copy(r_sb, o_ps)
            nc.sync.dma_start(out=res[m * 128 : (m + 1) * 128, cs], in_=r_sb)



def _standardize(x):
    mu = x.mean(-1, keepdims=True)
    var = ((x - mu) ** 2).mean(-1, keepdims=True)
    return ((x - mu) / np.sqrt(var + LN_EPS)).astype(np.float32)


def host_prepare(inputs):
    x = np.ascontiguousarray(np.asarray(inputs["x"], np.float32))
    g = np.asarray(inputs["ln_gamma"], np.float32)
    beta = np.asarray(inputs["ln_beta"], np.float32)
    W_in = np.asarray(inputs["W_in"], np.float32)
    conv_w = np.asarray(inputs["conv_w"], np.float32)[:, 0, :]
    conv_b = np.asarray(inputs["conv_b"], np.float32)
    W_out = np.asarray(inputs["W_out"], np.float32)
    A = np.asarray(inputs["A"], np.float32)
    Bm = np.asarray(inputs["Bm"], np.float32)
    Cm = np.asarray(inputs["Cm"], np.float32)

    Wg = W_in * g[None, :]
    b_in = W_in @ beta
    bias_u = b_in[:D_INNER]
    bias_z = b_in[D_INNER:]
    W1g = Wg[:D_INNER]

    Wvb0 = (Bm @ W_in[:D_INNER]) * g[None, :]
    bias_v0 = Bm @ W_in[:D_INNER] @ beta

    fallback = False
    lamc, V = np.linalg.eig(A.astype(np.float64))
    if np.abs(lamc.imag).max() > 1e-9 or np.linalg.cond(V) > 1e3:
        fallback = True
    if fallback:
        lam = np.zeros(D_STATE, np.float32)
        Wvb = np.zeros_like(Wvb0)
        Cmt = Cm.astype(np.float32)
        xn = _standardize(x.reshape(-1, D_MODEL)).reshape(x.shape) * g + beta
        v = xn.astype(np.float32) @ (Bm @ W_in[:D_INNER]).T
        sT = np.zeros((B, L, D_STATE), np.float32)
        for b_ in range(B):
            cur = np.zeros(D_STATE, np.float64)
            Ad = A.astype(np.float64)
            for tt in range(L):
                cur = Ad @ cur + v[b_, tt]
                sT[b_, tt] = cur
        sT = np.nan_to_num(sT, posinf=3e38, neginf=-3e38)
    else:
        lam = lamc.real
        Vr = V.real
        Vi = np.linalg.inv(Vr)
        Wvb = (Vi @ Wvb0).astype(np.float32)
        bias_vt = (Vi @ bias_v0).astype(np.float32)
        Cmt = (Vr.T @ Cm).astype(np.float32)

    wint = np.ascontiguousarray(
        Wg.reshape(FT, 128, KT, 128).transpose(0, 3, 2, 1))
    wvbt = np.ascontiguousarray(
        Wvb.reshape(D_STATE, KT, 128).transpose(2, 1, 0)) if not fallback \
        else np.zeros((128, KT, D_STATE), np.float32)
    wot = np.ascontiguousarray(
        W_out.reshape(MT, 128, CT, 128).transpose(0, 3, 2, 1)
    ).astype(ml_dtypes.bfloat16)
    convw_p = np.ascontiguousarray(conv_w.reshape(CT, 128, D_CONV).transpose(1, 0, 2))
    convb_f = conv_b + bias_u * conv_w.sum(axis=1)
    convb_p = np.ascontiguousarray(convb_f.reshape(CT, 128).T)
    biasz_p = np.ascontiguousarray(bias_z.reshape(CT, 128).T)
    decay_p = lam.astype(np.float32).reshape(D_STATE, 1)

    in_maps = []
    for c in range(N_CORES):
        b_, k = c // 4, c % 4
        xs = x[b_, k * TLOC : (k + 1) * TLOC]
        xTc = np.ascontiguousarray(xs.T)

        if k == 0:
            uh = np.zeros((D_INNER, 3), np.float32)
        else:
            xh = x[b_, k * TLOC - 3 : k * TLOC]
            uh = (_standardize(xh) @ W1g.T).T
        uh_p = np.ascontiguousarray(
            uh.reshape(CT, 128, 3).transpose(1, 0, 2)).astype(ml_dtypes.bfloat16)

        if fallback:
            va = np.ascontiguousarray(sT[b_, k * TLOC : (k + 1) * TLOC].T)
            G = np.zeros((N_CORES, D_STATE), np.float32)
        else:
            va = np.broadcast_to(bias_vt[:, None], (D_STATE, TLOC)).copy()
            G = np.zeros((N_CORES, D_STATE), np.float32)
            for j in range(N_CORES):
                bj, kj = j // 4, j % 4
                if bj == b_ and kj < k:
                    G[j] = lam ** (TLOC * (k - kj))
        in_maps.append(dict(
            xT=xTc, wint=wint, wvbt=wvbt, cmt=Cmt.astype(np.float32),
            wot=wot, convw=convw_p, convb=convb_p,
            biasz=biasz_p, uhalo=uh_p, vadd=va.astype(np.float32),
            decay=decay_p, gct=np.ascontiguousarray(G.T),
        ))
    return in_maps, x


def get_nc():
    global _NC_CACHE
    if _NC_CACHE is None:
        _NC_CACHE = build_graph()
    return _NC_CACHE


def kernel(**inputs):
    global LAST_RESULT
    nc = get_nc()
    in_maps, x = host_prepare(inputs)
    trace = bool(os.environ.get("BASS_TRACE"))
    r = run_bass_kernel_spmd(nc, in_maps, core_ids=list(range(N_CORES)),
                             trace=trace)
    LAST_RESULT = r
    out = np.empty((B, L, D_MODEL), np.float32)
    for c in range(N_CORES):
        b_, k = c // 4, c % 4
        resT = r.results[c]["res"]
        out[b_, k * TLOC : (k + 1) * TLOC] = (
            x[b_, k * TLOC : (k + 1) * TLOC] + resT.T)
    return out


# revision 63
# speedup vs baseline: 1.0220x; 1.0123x over previous
import os
import numpy as np
import ml_dtypes

import concourse.bass as bass
import concourse.tile as tile
from concourse import bacc
from concourse import mybir
from concourse.bass_utils import run_bass_kernel_spmd
from concourse.masks import make_identity

F32 = mybir.dt.float32
F32R = mybir.dt.float32r
BF16 = mybir.dt.bfloat16

D_MODEL, D_STATE, D_CONV = 1024, 16, 4
D_INNER = 2048
B, L = 2, 4096
LN_EPS = 1e-5
N_CORES = 8
TLOC = 1024
KT = D_MODEL // 128
FT = 2 * D_INNER // 128
CT = D_INNER // 128
MT = D_MODEL // 128
NCH = TLOC // 512

_NC_CACHE = None
LAST_RESULT = None


def r32(ap):
    return ap.bitcast(F32R)


def build_graph():
    nc = bacc.Bacc(num_devices=N_CORES)

    xT = nc.declare_dram_parameter("xT", [D_MODEL, TLOC], F32R, isOutput=False)
    wint = nc.declare_dram_parameter("wint", [FT, 128, KT, 128], F32R, isOutput=False)
    wvbt = nc.declare_dram_parameter("wvbt", [128, KT, D_STATE], F32R, isOutput=False)
    cmt = nc.declare_dram_parameter("cmt", [D_STATE, D_INNER], F32R, isOutput=False)
    wot = nc.declare_dram_parameter("wot", [MT, 128, CT, 128], BF16, isOutput=False)
    convw = nc.declare_dram_parameter("convw", [128, CT, D_CONV], F32, isOutput=False)
    convb = nc.declare_dram_parameter("convb", [128, CT], F32, isOutput=False)
    biasz = nc.declare_dram_parameter("biasz", [128, CT], F32, isOutput=False)
    uhalo = nc.declare_dram_parameter("uhalo", [128, CT, 3], BF16, isOutput=False)
    vadd = nc.declare_dram_parameter("vadd", [D_STATE, TLOC], F32, isOutput=False)
    decay = nc.declare_dram_parameter("decay", [D_STATE, 1], F32, isOutput=False)
    gct = nc.declare_dram_parameter("gct", [D_STATE, N_CORES], F32, isOutput=False)
    res = nc.declare_dram_parameter("res", [D_MODEL, TLOC], F32, isOutput=True)

    with tile.TileContext(nc) as tc:
        with (
            tc.tile_pool(name="sb", bufs=1) as sb,
            tc.tile_pool(name="sb2", bufs=2) as sb2,
            tc.tile_pool(name="ps", bufs=6, space="PSUM") as ps,
            tc.tile_pool(name="pss", bufs=2, space="PSUM") as pss,
            tc.tile_pool(name="dr", bufs=1, space="DRAM") as dr,
        ):
            with nc.allow_low_precision(reason="f32r matmul pipeline"):
                _emit(nc, tc, sb, sb2, ps, pss, dr, locals())
    nc.compile()
    return nc


def _emit(nc, tc, sb, sb2, ps, pss, dr, t):
    xT, wint, wvbt, cmt, wot = t["xT"], t["wint"], t["wvbt"], t["cmt"], t["wot"]
    convw, convb, biasz = t["convw"], t["convb"], t["biasz"]
    uhalo, vadd, decay, gct, res = t["uhalo"], t["vadd"], t["decay"], t["gct"], t["res"]

    x_sb = sb.tile([128, KT, TLOC], F32R)
    xTr = xT.rearrange("(ko ki) t -> ki ko t", ki=128)
    for n in range(NCH):
        for ko in range(KT):
            nc.sync.dma_start(out=x_sb[:, ko, n * 512 : (n + 1) * 512],
                              in_=xTr[:, ko, n * 512 : (n + 1) * 512])

    ones_k0 = sb.tile([128, 1], F32)
    nc.vector.memset(ones_k0, 1.0 / D_MODEL)
    ones_k = sb.tile([128, 1], F32R)
    nc.vector.tensor_copy(ones_k, ones_k0)
    ones_b = sb.tile([1, 128], F32)
    nc.vector.memset(ones_b, 1.0)
    eps_t = sb.tile([1, 1], F32)
    nc.vector.memset(eps_t, LN_EPS)
    ident = sb.tile([128, 128], BF16)
    make_identity(nc, ident)
    convw_sb = sb.tile([128, CT, D_CONV], F32)
    nc.sync.dma_start(out=convw_sb, in_=t["convw"][:])
    convb_sb = sb.tile([128, CT], F32)
    nc.sync.dma_start(out=convb_sb, in_=convb[:])
    biasz_sb = sb.tile([128, CT], F32)
    nc.sync.dma_start(out=biasz_sb, in_=biasz[:])
    rows = sb.tile([1, 2 * TLOC + 512], F32)
    states = sb.tile([D_STATE, 2 * TLOC], F32)
    s_sb = sb.tile([D_STATE, TLOC], F32R)
    vadd_sb = states[:, TLOC : 2 * TLOC]
    nc.sync.dma_start(out=vadd_sb, in_=vadd[:])
    decay_c = sb.tile([D_STATE, 1], F32)
    nc.sync.dma_start(out=decay_c, in_=decay[:])
    gct_sb = sb.tile([D_STATE, N_CORES], F32)
    nc.sync.dma_start(out=gct_sb, in_=gct[:])

    mu_row = rows[:, 0:TLOC]
    var_row = rows[:, TLOC : 2 * TLOC]
    mu2_row = rows[:, 2 * TLOC : 2 * TLOC + 512]
    r_row = mu2_row
    mur_row = var_row
    rb_sb = sb.tile([128, TLOC], F32)
    murb_sb = sb.tile([128, TLOC], F32)

    sq_half = []
    for n in range(NCH):
        cs = slice(n * 512, (n + 1) * 512)
        mu_ps = pss.tile([1, 512], F32, tag="sm", name=f"mu_ps{n}")
        sq_ps = pss.tile([1, 512], F32, tag="sm", name=f"sq_ps{n}")
        for ko in range(KT):
            sq_scr = sb2.tile([128, 512], F32R, name="sq_scr", bufs=2)
            nc.scalar.square(sq_scr, x_sb[:, ko, cs])
            nc.tensor.matmul(mu_ps, ones_k, x_sb[:, ko, cs],
                             start=(ko == 0), stop=(ko == KT - 1))
            nc.tensor.matmul(sq_ps, ones_k, sq_scr,
                             start=(ko == 0), stop=(ko == KT - 1))
        nc.vector.tensor_copy(mu_row[:, cs], mu_ps)
        nc.vector.tensor_mul(mu2_row, mu_row[:, cs], mu_ps)
        nc.vector.tensor_sub(var_row[:, cs], sq_ps, mu2_row)
        nc.scalar.activation(var_row[:, cs], var_row[:, cs],
                             mybir.ActivationFunctionType.Sqrt,
                             bias=eps_t, scale=1.0)
        nc.vector.reciprocal_approx_fast(mu2_row, var_row[:, cs])
        r_half = mu2_row
        nc.vector.tensor_mul(mur_row[:, cs], mu_row[:, cs], r_half)

        for srow, dst in ((r_half, rb_sb), (mur_row[:, cs], murb_sb)):
            b_ps = pss.tile([128, 512], F32, tag="sm", name="b_ps")
            nc.tensor.matmul(b_ps, ones_b, srow, start=True, stop=True)
            nc.vector.tensor_copy(dst[:, cs], b_ps)

        for ko in range(KT):
            nc.vector.tensor_mul(x_sb[:, ko, cs], x_sb[:, ko, cs], rb_sb[:, cs])
            nc.vector.tensor_sub(x_sb[:, ko, cs], x_sb[:, ko, cs], murb_sb[:, cs])

    wvb_sb = sb.tile([128, KT, D_STATE], F32R)
    nc.sync.dma_start(out=wvb_sb, in_=wvbt[:])
    v_sb = states[:, 0:TLOC]
    for n in range(NCH):
        cs = slice(n * 512, (n + 1) * 512)
        v_ps = pss.tile([D_STATE, 512], F32, tag="sm", name="v_ps")
        for ko in range(KT):
            nc.tensor.matmul(v_ps, wvb_sb[:, ko, :], x_sb[:, ko, cs],
                             start=(ko == 0), stop=(ko == KT - 1))
        nc.vector.tensor_add(v_sb[:, cs], v_ps, vadd_sb[:, cs])

    decay_t = decay_c.broadcast_to([D_STATE, TLOC])

    l_sb = vadd_sb
    nc.vector.tensor_tensor_scan(l_sb, decay_t, v_sb, 0.0,
                                 mybir.AluOpType.mult, mybir.AluOpType.add)

    cc_in = dr.tile([D_STATE, 1], F32)
    cc_out = dr.tile([D_STATE * N_CORES, 1], F32, addr_space="Shared")
    nc.sync.dma_start(out=cc_in[:], in_=l_sb[:, TLOC - 1 : TLOC])
    nc.gpsimd.collective_compute(
        "AllGather", mybir.AluOpType.bypass,
        replica_groups=[list(range(N_CORES))],
        ins=[cc_in[:]], outs=[cc_out[:]],
    )
    lam_all = sb.tile([D_STATE, N_CORES], F32)
    nc.sync.dma_start(out=lam_all,
                      in_=cc_out.rearrange("(j d) one -> d (j one)", d=D_STATE))
    sig_scr = sb.tile([D_STATE, N_CORES], F32)
    sigma = sb.tile([D_STATE, 1], F32)
    nc.vector.scalar_tensor_tensor(
        out=sig_scr, in0=lam_all, scalar=1.0, in1=gct_sb,
        op0=mybir.AluOpType.mult, op1=mybir.AluOpType.mult, accum_out=sigma)
    nc.vector.tensor_tensor_scan(s_sb, decay_t, v_sb, sigma,
                                 mybir.AluOpType.mult, mybir.AluOpType.add)

    u_sb = sb.tile([128, CT, TLOC + 3], BF16)
    nc.sync.dma_start(out=u_sb[:, :, 0:3], in_=uhalo[:])
    sigz_sb = sb.tile([128, CT, TLOC], BF16)
    for f in range(FT):
        wt = sb2.tile([128, KT, 128], F32R, name="wt", bufs=4)
        nc.sync.dma_start(out=wt, in_=wint[f])
        for n in range(NCH):
            cs = slice(n * 512, (n + 1) * 512)
            p_t = ps.tile([128, 512], F32, tag="mm", name=f"ip{f}_{n}")
            for ko in range(KT):
                nc.tensor.matmul(p_t, wt[:, ko, :], x_sb[:, ko, cs],
                                 start=(ko == 0), stop=(ko == KT - 1))
            if f < CT:
                nc.scalar.copy(
                    out=u_sb[:, f, 3 + n * 512 : 3 + (n + 1) * 512],
                    in_=p_t)
            else:
                c = f - CT
                nc.scalar.activation(
                    out=sigz_sb[:, c, n * 512 : (n + 1) * 512],
                    in_=p_t, func=mybir.ActivationFunctionType.Sigmoid,
                    bias=biasz_sb[:, c : c + 1], scale=1.0)

    cmt_sb = sb.tile([D_STATE, D_INNER], F32R)
    nc.sync.dma_start(out=cmt_sb, in_=cmt[:])
    y_sb = sb.tile([128, CT, TLOC], BF16)
    for c in range(CT):
        acc = sb2.tile([128, TLOC], F32, name="cacc", bufs=2)
        nc.vector.tensor_scalar_mul(
            out=acc, in0=u_sb[:, c, 0:TLOC], scalar1=convw_sb[:, c, 0:1])
        for tap in range(1, D_CONV):
            nc.vector.scalar_tensor_tensor(
                out=acc, in0=u_sb[:, c, tap : tap + TLOC],
                scalar=convw_sb[:, c, tap : tap + 1], in1=acc,
                op0=mybir.AluOpType.mult, op1=mybir.AluOpType.add)
        nc.scalar.activation(
            out=u_sb[:, c, 3 : 3 + TLOC], in_=acc,
            func=mybir.ActivationFunctionType.Silu,
            bias=convb_sb[:, c : c + 1], scale=1.0)
        for n in range(NCH):
            cs = slice(n * 512, (n + 1) * 512)
            sc_ps = ps.tile([128, 512], F32, tag="mm", name=f"sc{c}_{n}")
            nc.tensor.matmul(sc_ps, cmt_sb[:, c * 128 : (c + 1) * 128],
                             s_sb[:, cs], start=True, stop=False)
            nc.tensor.matmul(sc_ps, ident,
                             u_sb[:, c, 3 + n * 512 : 3 + (n + 1) * 512],
                             start=False, stop=True)
            nc.vector.tensor_mul(y_sb[:, c, cs], sc_ps, sigz_sb[:, c, cs])

    for m in range(MT):
        wo = sb2.tile([128, CT, 128], BF16, name="wo", bufs=2)
        nc.sync.dma_start(out=wo, in_=wot[m])
        for n in range(NCH):
            cs = slice(n * 512, (n + 1) * 512)
            o_ps = ps.tile([128, 512], F32, tag="mm", name=f"op{m}_{n}")
            for c in range(CT):
                nc.tensor.matmul(o_ps, wo[:, c, :], y_sb[:, c, cs],
                                 start=(c == 0), stop=(c == CT - 1))
            r_sb = sb2.tile([128, 512], F32, name="r_sb", bufs=1)
            nc.scalar.copy(r_sb, o_ps)
            nc.sync.dma_start(out=res[m * 128 : (m + 1) * 128, cs], in_=r_sb)



def _standardize(x):
    mu = x.mean(-1, keepdims=True)
    var = ((x - mu) ** 2).mean(-1, keepdims=True)
    return ((x - mu) / np.sqrt(var + LN_EPS)).astype(np.float32)


def host_prepare(inputs):
    x = np.ascontiguousarray(np.asarray(inputs["x"], np.float32))
    g = np.asarray(inputs["ln_gamma"], np.float32)
    beta = np.asarray(inputs["ln_beta"], np.float32)
    W_in = np.asarray(inputs["W_in"], np.float32)
    conv_w = np.asarray(inputs["conv_w"], np.float32)[:, 0, :]
    conv_b = np.asarray(inputs["conv_b"], np.float32)
    W_out = np.asarray(inputs["W_out"], np.float32)
    A = np.asarray(inputs["A"], np.float32)
    Bm = np.asarray(inputs["Bm"], np.float32)
    Cm = np.asarray(inputs["Cm"], np.float32)

    Wg = W_in * g[None, :]
    b_in = W_in @ beta
    bias_u = b_in[:D_INNER]
    bias_z = b_in[D_INNER:]
    W1g = Wg[:D_INNER]

    Wvb0 = (Bm @ W_in[:D_INNER]) * g[None, :]
    bias_v0 = Bm @ W_in[:D_INNER] @ beta

    fallback = False
    lamc, V = np.linalg.eig(A.astype(np.float64))
    if np.abs(lamc.imag).max() > 1e-9 or np.linalg.cond(V) > 1e3:
        fallback = True
    if fallback:
        lam = np.zeros(D_STATE, np.float32)
        Wvb = np.zeros_like(Wvb0)
        Cmt = Cm.astype(np.float32)
        xn = _standardize(x.reshape(-1, D_MODEL)).reshape(x.shape) * g + beta
        v = xn.astype(np.float32) @ (Bm @ W_in[:D_INNER]).T
        sT = np.zeros((B, L, D_STATE), np.float32)
        for b_ in range(B):
            cur = np.zeros(D_STATE, np.float64)
            Ad = A.astype(np.float64)
            for tt in range(L):
                cur = Ad @ cur + v[b_, tt]
                sT[b_, tt] = cur
        sT = np.nan_to_num(sT, posinf=3e38, neginf=-3e38)
    else:
        lam = lamc.real
        Vr = V.real
        Vi = np.linalg.inv(Vr)
        Wvb = (Vi @ Wvb0).astype(np.float32)
        bias_vt = (Vi @ bias_v0).astype(np.float32)
        Cmt = (Vr.T @ Cm).astype(np.float32)

    wint = np.ascontiguousarray(
        Wg.reshape(FT, 128, KT, 128).transpose(0, 3, 2, 1))
    wvbt = np.ascontiguousarray(
        Wvb.reshape(D_STATE, KT, 128).transpose(2, 1, 0)) if not fallback \
        else np.zeros((128, KT, D_STATE), np.float32)
    wot = np.ascontiguousarray(
        W_out.reshape(MT, 128, CT, 128).transpose(0, 3, 2, 1)
    ).astype(ml_dtypes.bfloat16)
    convw_p = np.ascontiguousarray(conv_w.reshape(CT, 128, D_CONV).transpose(1, 0, 2))
    convb_f = conv_b + bias_u * conv_w.sum(axis=1)
    convb_p = np.ascontiguousarray(convb_f.reshape(CT, 128).T)
    biasz_p = np.ascontiguousarray(bias_z.reshape(CT, 128).T)
    decay_p = lam.astype(np.float32).reshape(D_STATE, 1)

    in_maps = []
    for c in range(N_CORES):
        b_, k = c // 4, c % 4
        xs = x[b_, k * TLOC : (k + 1) * TLOC]
        xTc = np.ascontiguousarray(xs.T)

        if k == 0:
            uh = np.zeros((D_INNER, 3), np.float32)
        else:
            xh = x[b_, k * TLOC - 3 : k * TLOC]
            uh = (_standardize(xh) @ W1g.T).T
        uh_p = np.ascontiguousarray(
            uh.reshape(CT, 128, 3).transpose(1, 0, 2)).astype(ml_dtypes.bfloat16)

        if fallback:
            va = np.ascontiguousarray(sT[b_, k * TLOC : (k + 1) * TLOC].T)
            G = np.zeros((N_CORES, D_STATE), np.float32)
        else:
            va = np.broadcast_to(bias_vt[:, None], (D_STATE, TLOC)).copy()
            G = np.zeros((N_CORES, D_STATE), np.float32)
            for j in range(N_CORES):
                bj, kj = j // 4, j % 4
                if bj == b_ and kj < k:
                    G[j] = lam ** (TLOC * (k - kj))
        in_maps.append(dict(
            xT=xTc, wint=wint, wvbt=wvbt, cmt=Cmt.astype(np.float32),
            wot=wot, convw=convw_p, convb=convb_p,
            biasz=biasz_p, uhalo=uh_p, vadd=va.astype(np.float32),
            decay=decay_p, gct=np.ascontiguousarray(G.T),
        ))
    return in_maps, x


def get_nc():
    global _NC_CACHE
    if _NC_CACHE is None:
        _NC_CACHE = build_graph()
    return _NC_CACHE


def kernel(**inputs):
    global LAST_RESULT
    nc = get_nc()
    in_maps, x = host_prepare(inputs)
    trace = bool(os.environ.get("BASS_TRACE"))
    r = run_bass_kernel_spmd(nc, in_maps, core_ids=list(range(N_CORES)),
                             trace=trace)
    LAST_RESULT = r
    out = np.empty((B, L, D_MODEL), np.float32)
    for c in range(N_CORES):
        b_, k = c // 4, c % 4
        resT = r.results[c]["res"]
        out[b_, k * TLOC : (k + 1) * TLOC] = (
            x[b_, k * TLOC : (k + 1) * TLOC] + resT.T)
    return out


# revision 66
# speedup vs baseline: 1.0309x; 1.0088x over previous
import os
import numpy as np
import ml_dtypes

import concourse.bass as bass
import concourse.tile as tile
from concourse import bacc
from concourse import mybir
from concourse.bass_utils import run_bass_kernel_spmd
from concourse.masks import make_identity

F32 = mybir.dt.float32
F32R = mybir.dt.float32r
BF16 = mybir.dt.bfloat16

D_MODEL, D_STATE, D_CONV = 1024, 16, 4
D_INNER = 2048
B, L = 2, 4096
LN_EPS = 1e-5
N_CORES = 8
TLOC = 1024
KT = D_MODEL // 128
FT = 2 * D_INNER // 128
CT = D_INNER // 128
MT = D_MODEL // 128
NCH = TLOC // 512

_NC_CACHE = None
LAST_RESULT = None


def r32(ap):
    return ap.bitcast(F32R)


def build_graph():
    nc = bacc.Bacc(num_devices=N_CORES)

    xT = nc.declare_dram_parameter("xT", [D_MODEL, TLOC], F32R, isOutput=False)
    wint = nc.declare_dram_parameter("wint", [FT, 128, KT, 128], F32R, isOutput=False)
    wvbt = nc.declare_dram_parameter("wvbt", [128, KT, D_STATE], F32R, isOutput=False)
    cmt = nc.declare_dram_parameter("cmt", [D_STATE, D_INNER], F32R, isOutput=False)
    wot = nc.declare_dram_parameter("wot", [MT, 128, CT, 128], BF16, isOutput=False)
    convw = nc.declare_dram_parameter("convw", [128, CT, D_CONV], F32, isOutput=False)
    convb = nc.declare_dram_parameter("convb", [128, CT], F32, isOutput=False)
    biasz = nc.declare_dram_parameter("biasz", [128, CT], F32, isOutput=False)
    uhalo = nc.declare_dram_parameter("uhalo", [128, CT, 3], BF16, isOutput=False)
    vadd = nc.declare_dram_parameter("vadd", [D_STATE, TLOC], F32, isOutput=False)
    decay = nc.declare_dram_parameter("decay", [D_STATE, 1], F32, isOutput=False)
    gct = nc.declare_dram_parameter("gct", [D_STATE, N_CORES], F32, isOutput=False)
    res = nc.declare_dram_parameter("res", [D_MODEL, TLOC], F32, isOutput=True)

    with tile.TileContext(nc) as tc:
        with (
            tc.tile_pool(name="sb", bufs=1) as sb,
            tc.tile_pool(name="sb2", bufs=2) as sb2,
            tc.tile_pool(name="ps", bufs=6, space="PSUM") as ps,
            tc.tile_pool(name="pss", bufs=2, space="PSUM") as pss,
            tc.tile_pool(name="dr", bufs=1, space="DRAM") as dr,
        ):
            with nc.allow_low_precision(reason="f32r matmul pipeline"):
                _emit(nc, tc, sb, sb2, ps, pss, dr, locals())
    nc.compile()
    return nc


def _emit(nc, tc, sb, sb2, ps, pss, dr, t):
    xT, wint, wvbt, cmt, wot = t["xT"], t["wint"], t["wvbt"], t["cmt"], t["wot"]
    convw, convb, biasz = t["convw"], t["convb"], t["biasz"]
    uhalo, vadd, decay, gct, res = t["uhalo"], t["vadd"], t["decay"], t["gct"], t["res"]

    x_sb = sb.tile([128, KT, TLOC], F32R)
    xTr = xT.rearrange("(ko ki) t -> ki ko t", ki=128)
    for n in range(NCH):
        for ko in range(KT):
            nc.sync.dma_start(out=x_sb[:, ko, n * 512 : (n + 1) * 512],
                              in_=xTr[:, ko, n * 512 : (n + 1) * 512])

    ones_k0 = sb.tile([128, 1], F32)
    nc.vector.memset(ones_k0, 1.0 / D_MODEL)
    ones_k = sb.tile([128, 1], F32R)
    nc.vector.tensor_copy(ones_k, ones_k0)
    ones_b0 = sb.tile([1, 128], F32)
    nc.vector.memset(ones_b0, 1.0)
    ones_b = sb.tile([1, 128], F32R)
    nc.vector.tensor_copy(ones_b, ones_b0)
    rr = sb.tile([1, 1024], F32R)
    eps_t = sb.tile([1, 1], F32)
    nc.vector.memset(eps_t, LN_EPS)
    ident = sb.tile([128, 128], BF16)
    make_identity(nc, ident)
    convw_sb = sb.tile([128, CT, D_CONV], F32)
    nc.sync.dma_start(out=convw_sb, in_=t["convw"][:])
    convb_sb = sb.tile([128, CT], F32)
    nc.sync.dma_start(out=convb_sb, in_=convb[:])
    biasz_sb = sb.tile([128, CT], F32)
    nc.sync.dma_start(out=biasz_sb, in_=biasz[:])
    rows = sb.tile([1, 3 * 512], F32)
    states = sb.tile([D_STATE, 2 * TLOC], F32)
    s_sb = sb.tile([D_STATE, TLOC], F32R)
    vadd_sb = states[:, TLOC : 2 * TLOC]
    nc.sync.dma_start(out=vadd_sb, in_=vadd[:])
    decay_c = sb.tile([D_STATE, 1], F32)
    nc.sync.dma_start(out=decay_c, in_=decay[:])
    gct_sb = sb.tile([D_STATE, N_CORES], F32)
    nc.sync.dma_start(out=gct_sb, in_=gct[:])

    mu_row = rows[:, 0:512]
    var_row = rows[:, 512:1024]
    mu2_row = rows[:, 1024:1536]
    rb_sb = sb.tile([128, TLOC], F32)
    murb_sb = sb.tile([128, TLOC], F32)

    sq_half = []
    for n in range(NCH):
        cs = slice(n * 512, (n + 1) * 512)
        mu_ps = pss.tile([1, 512], F32, tag="sm", name=f"mu_ps{n}")
        sq_ps = pss.tile([1, 512], F32, tag="sm", name=f"sq_ps{n}")
        for ko in range(KT):
            sq_scr = sb2.tile([128, 512], F32R, name="sq_scr", bufs=2)
            nc.scalar.square(sq_scr, x_sb[:, ko, cs])
            nc.tensor.matmul(mu_ps, ones_k, x_sb[:, ko, cs],
                             start=(ko == 0), stop=(ko == KT - 1))
            nc.tensor.matmul(sq_ps, ones_k, sq_scr,
                             start=(ko == 0), stop=(ko == KT - 1))
        nc.vector.tensor_copy(mu_row, mu_ps)
        nc.vector.tensor_mul(mu2_row, mu_row, mu_ps)
        nc.vector.tensor_sub(var_row, sq_ps, mu2_row)
        nc.scalar.activation(var_row, var_row,
                             mybir.ActivationFunctionType.Sqrt,
                             bias=eps_t, scale=1.0)
        nc.vector.reciprocal_approx_fast(mu2_row, var_row)
        r_half = mu2_row
        rr_r, rr_mur = rr[:, 0:512], rr[:, 512:1024]
        nc.vector.tensor_copy(rr_r, r_half)
        nc.vector.tensor_mul(rr_mur, mu_row, r_half)

        for srow, dst in ((rr_r, rb_sb), (rr_mur, murb_sb)):
            b_ps = pss.tile([128, 512], F32, tag="sm", name="b_ps")
            nc.tensor.matmul(b_ps, ones_b, srow, start=True, stop=True)
            nc.vector.tensor_copy(dst[:, cs], b_ps)

        for ko in range(KT):
            nc.vector.tensor_mul(x_sb[:, ko, cs], x_sb[:, ko, cs], rb_sb[:, cs])
            nc.vector.tensor_sub(x_sb[:, ko, cs], x_sb[:, ko, cs], murb_sb[:, cs])

    wvb_sb = sb.tile([128, KT, D_STATE], F32R)
    nc.sync.dma_start(out=wvb_sb, in_=wvbt[:])
    v_sb = states[:, 0:TLOC]
    for n in range(NCH):
        cs = slice(n * 512, (n + 1) * 512)
        v_ps = pss.tile([D_STATE, 512], F32, tag="sm", name="v_ps")
        for ko in range(KT):
            nc.tensor.matmul(v_ps, wvb_sb[:, ko, :], x_sb[:, ko, cs],
                             start=(ko == 0), stop=(ko == KT - 1))
        nc.vector.tensor_add(v_sb[:, cs], v_ps, vadd_sb[:, cs])

    decay_t = decay_c.broadcast_to([D_STATE, TLOC])

    l_sb = vadd_sb
    nc.vector.tensor_tensor_scan(l_sb, decay_t, v_sb, 0.0,
                                 mybir.AluOpType.mult, mybir.AluOpType.add)

    cc_in = dr.tile([D_STATE, 1], F32)
    cc_out = dr.tile([D_STATE * N_CORES, 1], F32, addr_space="Shared")
    nc.sync.dma_start(out=cc_in[:], in_=l_sb[:, TLOC - 1 : TLOC])
    nc.gpsimd.collective_compute(
        "AllGather", mybir.AluOpType.bypass,
        replica_groups=[list(range(N_CORES))],
        ins=[cc_in[:]], outs=[cc_out[:]],
    )
    lam_all = sb.tile([D_STATE, N_CORES], F32)
    nc.sync.dma_start(out=lam_all,
                      in_=cc_out.rearrange("(j d) one -> d (j one)", d=D_STATE))
    sig_scr = sb.tile([D_STATE, N_CORES], F32)
    sigma = sb.tile([D_STATE, 1], F32)
    nc.vector.scalar_tensor_tensor(
        out=sig_scr, in0=lam_all, scalar=1.0, in1=gct_sb,
        op0=mybir.AluOpType.mult, op1=mybir.AluOpType.mult, accum_out=sigma)
    nc.vector.tensor_tensor_scan(s_sb, decay_t, v_sb, sigma,
                                 mybir.AluOpType.mult, mybir.AluOpType.add)

    u_sb = sb.tile([128, CT, TLOC + 3], BF16)
    nc.sync.dma_start(out=u_sb[:, :, 0:3], in_=uhalo[:])
    sigz_sb = sb.tile([128, CT, TLOC], BF16)
    for f in range(FT):
        wt = sb2.tile([128, KT, 128], F32R, name="wt", bufs=4)
        nc.sync.dma_start(out=wt, in_=wint[f])
        for n in range(NCH):
            cs = slice(n * 512, (n + 1) * 512)
            p_t = ps.tile([128, 512], F32, tag="mm", name=f"ip{f}_{n}")
            for ko in range(KT):
                nc.tensor.matmul(p_t, wt[:, ko, :], x_sb[:, ko, cs],
                                 start=(ko == 0), stop=(ko == KT - 1))
            if f < CT:
                nc.scalar.copy(
                    out=u_sb[:, f, 3 + n * 512 : 3 + (n + 1) * 512],
                    in_=p_t)
            else:
                c = f - CT
                nc.scalar.activation(
                    out=sigz_sb[:, c, n * 512 : (n + 1) * 512],
                    in_=p_t, func=mybir.ActivationFunctionType.Sigmoid,
                    bias=biasz_sb[:, c : c + 1], scale=1.0)

    cmt_sb = sb.tile([D_STATE, D_INNER], F32R)
    nc.sync.dma_start(out=cmt_sb, in_=cmt[:])
    y_sb = sb.tile([128, CT, TLOC], BF16)
    for c in range(CT):
        acc = sb2.tile([128, TLOC], F32, name="cacc", bufs=2)
        nc.vector.tensor_scalar_mul(
            out=acc, in0=u_sb[:, c, 0:TLOC], scalar1=convw_sb[:, c, 0:1])
        for tap in range(1, D_CONV):
            nc.vector.scalar_tensor_tensor(
                out=acc, in0=u_sb[:, c, tap : tap + TLOC],
                scalar=convw_sb[:, c, tap : tap + 1], in1=acc,
                op0=mybir.AluOpType.mult, op1=mybir.AluOpType.add)
        nc.scalar.activation(
            out=u_sb[:, c, 3 : 3 + TLOC], in_=acc,
            func=mybir.ActivationFunctionType.Silu,
            bias=convb_sb[:, c : c + 1], scale=1.0)
        for n in range(NCH):
            cs = slice(n * 512, (n + 1) * 512)
            sc_ps = ps.tile([128, 512], F32, tag="mm", name=f"sc{c}_{n}")
            nc.tensor.matmul(sc_ps, cmt_sb[:, c * 128 : (c + 1) * 128],
                             s_sb[:, cs], start=True, stop=False)
            nc.tensor.matmul(sc_ps, ident,
                             u_sb[:, c, 3 + n * 512 : 3 + (n + 1) * 512],
                             start=False, stop=True)
            nc.vector.tensor_mul(y_sb[:, c, cs], sc_ps, sigz_sb[:, c, cs])

    for m in range(MT):
        wo = sb2.tile([128, CT, 128], BF16, name="wo", bufs=2)
        nc.sync.dma_start(out=wo, in_=wot[m])
        for n in range(NCH):
            cs = slice(n * 512, (n + 1) * 512)
            o_ps = ps.tile([128, 512], F32, tag="mm", name=f"op{m}_{n}")
            for c in range(CT):
                nc.tensor.matmul(o_ps, wo[:, c, :], y_sb[:, c, cs],
                                 start=(c == 0), stop=(c == CT - 1))
            r_sb = sb2.tile([128, 512], F32, name="r_sb", bufs=1)
            nc.scalar.copy(r_sb, o_ps)
            nc.sync.dma_start(out=res[m * 128 : (m + 1) * 128, cs], in_=r_sb)



def _standardize(x):
    mu = x.mean(-1, keepdims=True)
    var = ((x - mu) ** 2).mean(-1, keepdims=True)
    return ((x - mu) / np.sqrt(var + LN_EPS)).astype(np.float32)


def host_prepare(inputs):
    x = np.ascontiguousarray(np.asarray(inputs["x"], np.float32))
    g = np.asarray(inputs["ln_gamma"], np.float32)
    beta = np.asarray(inputs["ln_beta"], np.float32)
    W_in = np.asarray(inputs["W_in"], np.float32)
    conv_w = np.asarray(inputs["conv_w"], np.float32)[:, 0, :]
    conv_b = np.asarray(inputs["conv_b"], np.float32)
    W_out = np.asarray(inputs["W_out"], np.float32)
    A = np.asarray(inputs["A"], np.float32)
    Bm = np.asarray(inputs["Bm"], np.float32)
    Cm = np.asarray(inputs["Cm"], np.float32)

    Wg = W_in * g[None, :]
    b_in = W_in @ beta
    bias_u = b_in[:D_INNER]
    bias_z = b_in[D_INNER:]
    W1g = Wg[:D_INNER]

    Wvb0 = (Bm @ W_in[:D_INNER]) * g[None, :]
    bias_v0 = Bm @ W_in[:D_INNER] @ beta

    fallback = False
    lamc, V = np.linalg.eig(A.astype(np.float64))
    if np.abs(lamc.imag).max() > 1e-9 or np.linalg.cond(V) > 1e3:
        fallback = True
    if fallback:
        lam = np.zeros(D_STATE, np.float32)
        Wvb = np.zeros_like(Wvb0)
        Cmt = Cm.astype(np.float32)
        xn = _standardize(x.reshape(-1, D_MODEL)).reshape(x.shape) * g + beta
        v = xn.astype(np.float32) @ (Bm @ W_in[:D_INNER]).T
        sT = np.zeros((B, L, D_STATE), np.float32)
        for b_ in range(B):
            cur = np.zeros(D_STATE, np.float64)
            Ad = A.astype(np.float64)
            for tt in range(L):
                cur = Ad @ cur + v[b_, tt]
                sT[b_, tt] = cur
        sT = np.nan_to_num(sT, posinf=3e38, neginf=-3e38)
    else:
        lam = lamc.real
        Vr = V.real
        Vi = np.linalg.inv(Vr)
        Wvb = (Vi @ Wvb0).astype(np.float32)
        bias_vt = (Vi @ bias_v0).astype(np.float32)
        Cmt = (Vr.T @ Cm).astype(np.float32)

    wint = np.ascontiguousarray(
        Wg.reshape(FT, 128, KT, 128).transpose(0, 3, 2, 1))
    wvbt = np.ascontiguousarray(
        Wvb.reshape(D_STATE, KT, 128).transpose(2, 1, 0)) if not fallback \
        else np.zeros((128, KT, D_STATE), np.float32)
    wot = np.ascontiguousarray(
        W_out.reshape(MT, 128, CT, 128).transpose(0, 3, 2, 1)
    ).astype(ml_dtypes.bfloat16)
    convw_p = np.ascontiguousarray(conv_w.reshape(CT, 128, D_CONV).transpose(1, 0, 2))
    convb_f = conv_b + bias_u * conv_w.sum(axis=1)
    convb_p = np.ascontiguousarray(convb_f.reshape(CT, 128).T)
    biasz_p = np.ascontiguousarray(bias_z.reshape(CT, 128).T)
    decay_p = lam.astype(np.float32).reshape(D_STATE, 1)

    in_maps = []
    for c in range(N_CORES):
        b_, k = c // 4, c % 4
        xs = x[b_, k * TLOC : (k + 1) * TLOC]
        xTc = np.ascontiguousarray(xs.T)

        if k == 0:
            uh = np.zeros((D_INNER, 3), np.float32)
        else:
            xh = x[b_, k * TLOC - 3 : k * TLOC]
            uh = (_standardize(xh) @ W1g.T).T
        uh_p = np.ascontiguousarray(
            uh.reshape(CT, 128, 3).transpose(1, 0, 2)).astype(ml_dtypes.bfloat16)

        if fallback:
            va = np.ascontiguousarray(sT[b_, k * TLOC : (k + 1) * TLOC].T)
            G = np.zeros((N_CORES, D_STATE), np.float32)
        else:
            va = np.broadcast_to(bias_vt[:, None], (D_STATE, TLOC)).copy()
            G = np.zeros((N_CORES, D_STATE), np.float32)
            for j in range(N_CORES):
                bj, kj = j // 4, j % 4
                if bj == b_ and kj < k:
                    G[j] = lam ** (TLOC * (k - kj))
        in_maps.append(dict(
            xT=xTc, wint=wint, wvbt=wvbt, cmt=Cmt.astype(np.float32),
            wot=wot, convw=convw_p, convb=convb_p,
            biasz=biasz_p, uhalo=uh_p, vadd=va.astype(np.float32),
            decay=decay_p, gct=np.ascontiguousarray(G.T),
        ))
    return in_maps, x


def get_nc():
    global _NC_CACHE
    if _NC_CACHE is None:
        _NC_CACHE = build_graph()
    return _NC_CACHE


def kernel(**inputs):
    global LAST_RESULT
    nc = get_nc()
    in_maps, x = host_prepare(inputs)
    trace = bool(os.environ.get("BASS_TRACE"))
    r = run_bass_kernel_spmd(nc, in_maps, core_ids=list(range(N_CORES)),
                             trace=trace)
    LAST_RESULT = r
    out = np.empty((B, L, D_MODEL), np.float32)
    for c in range(N_CORES):
        b_, k = c // 4, c % 4
        resT = r.results[c]["res"]
        out[b_, k * TLOC : (k + 1) * TLOC] = (
            x[b_, k * TLOC : (k + 1) * TLOC] + resT.T)
    return out
